# revision 17
# baseline (speedup 1.0000x reference)
"""DiT block kernel for Trainium2, SPMD data-parallel over batch across 8 NeuronCores.

Per-core computation (one batch element, N=1024 tokens, D=1024):
  adaLN1 -> qkv -> attention(16 heads, hd=64) -> proj + residual
  adaLN2 -> fc1 -> gelu(exact/erf) -> fc2 + residual

Layout strategy:
  - residual stream x kept token-major (tm) [tok_p, feat] fp32 in SBUF
  - LN stats via bn_stats along free dim; adaLN scale/shift broadcast across
    partitions via tiny PE ones-matmuls
  - normalized h transposed to feature-major (fm) bf16 via PE transpose
  - all big matmuls in bf16 (fp32 PSUM accumulation)
  - Q^T,K^T produced fm (weights stationary), V token-major (acts stationary)
    with a ones-column appended so the attention AV matmul also produces the
    softmax denominators (softmax computed without max subtraction - safe at
    these scales, exact same math as reference)
  - proj/fc2 act-stationary -> token-major outputs + fp32 residual adds
"""

import sys

if "/opt/trn_rl_repo" not in sys.path:
    sys.path.insert(0, "/opt/trn_rl_repo")

from contextlib import ExitStack

import ml_dtypes
import numpy as np

import concourse.bacc as bacc
import concourse.bass as bass
import concourse.mybir as mybir
import concourse.tile as tile
from concourse.bass import ds, ts
from concourse.masks import make_identity

FP32 = mybir.dt.float32
BF16 = mybir.dt.bfloat16
AF = mybir.ActivationFunctionType
ALU = mybir.AluOpType

B, N, D = 8, 1024, 1024
H, HD, DFF = 16, 64, 4096
P = 128
NT = N // P   # 8 token tiles
KT = D // P   # 8 feature k-tiles
EPS = 1e-6
# "erf": exact gelu via Erf activation (not implemented in CoreSim, HW ok)
# "tanh": tanh-approx gelu from Square+Tanh (CoreSim-compatible fallback)
GELU_MODE = "erf"

BF16_NP = ml_dtypes.bfloat16


def build():
    """Build the single-core program (same program on all 8 cores)."""
    nc = bacc.Bacc(None, target_bir_lowering=False, debug=False)
    names = {}

    with tile.TileContext(nc) as tc:
        with ExitStack() as root:
            dram = root.enter_context(tc.tile_pool(name="dram", bufs=1, space="DRAM"))

            def din(nm, shape, dt=BF16):
                t = dram.tile(shape, dt, kind="ExternalInput", name=nm)
                names[nm] = t.name
                return t

            x_d = din("x", [N, D], FP32)
            condt_d = din("condt", [P, KT])
            wqk_d = din("wqk", [KT, 16, P, P])
            wv_d = din("wv", [KT, 2, P, 512])
            wproj_d = din("wproj", [KT, 2, P, 512])
            wada1_d = din("wada1", [KT, 4, P, 512])
            wada2_d = din("wada2", [KT, 4, P, 512])
            wfc1_d = din("wfc1", [KT, 32, P, P])
            wfc2_d = din("wfc2", [32, 2, P, 512])
            bada1_d = din("bada1", [1, 2 * D], FP32)
            bada2_d = din("bada2", [1, 2 * D], FP32)
            bqt_d = din("bqt", [P, KT], FP32)
            bkt_d = din("bkt", [P, KT], FP32)
            bvt_d = din("bvt", [P, KT], FP32)
            bfc1t_d = din("bfc1t", [P, 32], FP32)
            bfc1ts_d = din("bfc1ts", [P, 32], FP32)
            bproj_d = din("bprojbf", [1, D])
            bfc2_d = din("bfc2bf", [1, D])
            out_d = dram.tile([N, D], FP32, kind="ExternalOutput", name="out")
            names["out"] = out_d.name

            # ---------------- constants / small inputs ----------------
            const = root.enter_context(tc.tile_pool(name="const", bufs=1))
            psum = root.enter_context(tc.tile_pool(name="psum", bufs=6, space="PSUM"))

            def pt(nm="ps"):
                return psum.tile([P, 512], FP32, tag="ps", name=nm, bufs=4)

            def pav(nm="pav"):
                return psum.tile([P, 512], FP32, tag="pav", name=nm, bufs=2)

            def pt_tr(nm="pstr"):
                return psum.tile([P, P], BF16, tag="pstr", name=nm, bufs=2)

            condt_sb = const.tile([P, KT], BF16, name="condt_sb")
            nc.sync.dma_start(out=condt_sb[:, :], in_=condt_d[:, :])
            bqt_sb = const.tile([P, KT], FP32, name="bqt_sb")
            nc.sync.dma_start(out=bqt_sb[:, :], in_=bqt_d[:, :])
            bkt_sb = const.tile([P, KT], FP32, name="bkt_sb")
            nc.sync.dma_start(out=bkt_sb[:, :], in_=bkt_d[:, :])
            bvt_sb = const.tile([P, KT], FP32, name="bvt_sb")
            nc.sync.dma_start(out=bvt_sb[:, :], in_=bvt_d[:, :])
            bfc1t_sb = const.tile([P, 32], FP32, name="bfc1t_sb")
            nc.sync.dma_start(out=bfc1t_sb[:, :], in_=bfc1t_d[:, :])
            bfc1ts_sb = const.tile([P, 32], FP32, name="bfc1ts_sb")
            nc.sync.dma_start(out=bfc1ts_sb[:, :], in_=bfc1ts_d[:, :])
            bproj_sb = const.tile([1, D], BF16, name="bproj_sb")
            nc.sync.dma_start(out=bproj_sb[:, :], in_=bproj_d[:, :])
            bfc2_sb = const.tile([1, D], BF16, name="bfc2_sb")
            nc.sync.dma_start(out=bfc2_sb[:, :], in_=bfc2_d[:, :])
            bada_sb = []
            for ai, bd in enumerate((bada1_d, bada2_d)):
                t = const.tile([1, 2 * D], FP32, name=f"bada{ai}_sb")
                nc.sync.dma_start(out=t[:, :], in_=bd[:, :])
                bada_sb.append(t)

            ones_bf = const.tile([1, P], BF16, name="ones_bf")
            nc.vector.memset(ones_bf[:, :], 1.0)
            ident_bf = const.tile([P, P], BF16, name="ident_bf")
            make_identity(nc, ident_bf[:, :])
            zero_col = const.tile([P, 1], FP32, name="zero_col")
            nc.vector.memset(zero_col[:, :], 0.0)
            nc.const_aps.aps[(FP32, 0.0)] = zero_col[:, :]
            eps_col = const.tile([P, 1], FP32, name="eps_col")
            nc.vector.memset(eps_col[:, :], EPS)

            # ---------------- adaLN scale/shift rows ----------------
            ss_bf = []  # per ada: ((1+scale) bf16 row, shift bf16 row)

            def ada_block(ai, wada_d):
                with tc.tile_pool(name=f"wada{ai}", bufs=3) as wada_pool:
                    ss = const.tile([1, 2 * D], FP32, name=f"ss{ai}")
                    for og in range(4):
                        ps = pt()
                        for kt in range(KT):
                            wt = wada_pool.tile(
                                [P, 512], BF16, tag="wada", name="wadat"
                            )
                            nc.sync.dma_start(out=wt[:, :], in_=wada_d[kt, og])
                            nc.tensor.matmul(
                                ps[0:1, :],
                                lhsT=condt_sb[:, kt : kt + 1],
                                rhs=wt[:, :],
                                start=(kt == 0),
                                stop=(kt == KT - 1),
                            )
                        nc.vector.tensor_tensor(
                            out=ss[:, ds(og * 512, 512)],
                            in0=ps[0:1, :],
                            in1=bada_sb[ai][:, ds(og * 512, 512)],
                            op=ALU.add,
                        )
                    s_bf = const.tile([1, D], BF16, name=f"sbf{ai}")
                    nc.vector.tensor_scalar(
                        out=s_bf[:, :], in0=ss[:, 0:D], scalar1=1.0, scalar2=None,
                        op0=ALU.add,
                    )
                    sh_bf = const.tile([1, D], BF16, name=f"shbf{ai}")
                    nc.vector.tensor_copy(out=sh_bf[:, :], in_=ss[:, D : 2 * D])
                    ss_bf.append((s_bf, sh_bf))

            ada_block(0, wada1_d)

            def bcast_row(row_bf, nm, pool):
                """[1, D] bf16 row -> [128, D] fp32 (PE ones-matmul broadcast)."""
                outt = pool.tile([P, D], FP32, name=nm)
                for og in range(2):
                    ps = pt()
                    nc.tensor.matmul(
                        ps[:, :],
                        lhsT=ones_bf[0:1, :],
                        rhs=row_bf[0:1, ds(og * 512, 512)],
                        start=True,
                        stop=True,
                    )
                    nc.vector.tensor_copy(out=outt[:, ds(og * 512, 512)], in_=ps[:, :])
                return outt

            def ln_transpose(x_t, s_b, sh_b, hT, scr):
                """LayerNorm(x_t)*(s_b) + sh_b, then transpose to fm bf16 hT."""
                for tt in range(NT):
                    xt = x_t[:, tt, :]
                    st = scr.tile([P, 2, 6], FP32, tag="bnst", name="bnst")
                    xr = xt.rearrange("p (s f) -> p s f", f=512)
                    for sg in range(2):
                        nc.vector.bn_stats(out=st[:, sg, :], in_=xr[:, sg, :])
                    mv = scr.tile([P, 2], FP32, tag="bnmv", name="bnmv")
                    nc.vector.bn_aggr(out=mv[:, :], in_=st[:, :, :])
                    nc.scalar.activation(
                        out=mv[:, 1:2], in_=mv[:, 1:2], func=AF.Sqrt,
                        bias=eps_col[:, 0:1],
                    )
                    nc.vector.reciprocal(out=mv[:, 1:2], in_=mv[:, 1:2])
                    xn = scr.tile([P, D], FP32, tag="xn", name="xn")
                    nc.vector.tensor_scalar(
                        out=xn[:, :], in0=xt, scalar1=mv[:, 0:1], scalar2=mv[:, 1:2],
                        op0=ALU.subtract, op1=ALU.mult,
                    )
                    t2 = scr.tile([P, D], FP32, tag="t2", name="t2")
                    nc.vector.tensor_tensor(
                        out=t2[:, :], in0=xn[:, :], in1=s_b[:, :], op=ALU.mult
                    )
                    hbf = scr.tile([P, D], BF16, tag="hbf", name="hbf")
                    nc.vector.tensor_tensor(
                        out=hbf[:, :], in0=t2[:, :], in1=sh_b[:, :], op=ALU.add
                    )
                    for ft in range(KT):
                        ps = pt_tr()
                        nc.tensor.transpose(
                            ps[:, :], hbf[:, ts(ft, P)], ident_bf[:, :]
                        )
                        nc.vector.tensor_copy(
                            out=hT[:, ft, ts(tt, P)], in_=ps[:, :]
                        )

            # ---------------- load x ----------------
            es_x = ExitStack()
            p_x = es_x.enter_context(tc.tile_pool(name="p_x", bufs=1))
            x_sb = p_x.tile([P, NT, D], FP32, name="x_sb")
            for tt in range(NT):
                nc.sync.dma_start(out=x_sb[:, tt, :], in_=x_d[ts(tt, P), :])

            # ---------------- phase B: LN1 + transpose ----------------
            es_h1 = ExitStack()
            p_h1 = es_h1.enter_context(tc.tile_pool(name="p_h1", bufs=1))
            h1T = p_h1.tile([P, KT, N], BF16, name="h1T")
            with tc.tile_pool(name="lnscr1", bufs=2) as scr1, \
                 tc.tile_pool(name="sb1", bufs=1) as sb1:
                s1b = bcast_row(ss_bf[0][0], "s1b", sb1)
                sh1b = bcast_row(ss_bf[0][1], "sh1b", sb1)
                ln_transpose(x_sb, s1b, sh1b, h1T, scr1)

            # ---------------- phase C: QKV ----------------
            es_qkv = ExitStack()
            p_qkv = es_qkv.enter_context(tc.tile_pool(name="p_qkv", bufs=1))
            qT = p_qkv.tile([P, KT, N], BF16, name="qT")
            kTt = p_qkv.tile([P, KT, N], BF16, name="kTt")
            V1 = p_qkv.tile([P, NT, H, HD + 1], BF16, name="V1")
            nc.vector.memset(V1[:, :, :, HD : HD + 1], 1.0)

            with tc.tile_pool(name="wqk", bufs=3) as wqk_pool:
                for oft in range(16):
                    wt = wqk_pool.tile([P, KT, P], BF16, tag="wqk", name="wqkt")
                    for kt in range(KT):
                        nc.sync.dma_start(out=wt[:, kt, :], in_=wqk_d[kt, oft])
                    for tg in range(2):
                        ps = pt()
                        for kt in range(KT):
                            nc.tensor.matmul(
                                ps[:, :],
                                lhsT=wt[:, kt, :],
                                rhs=h1T[:, kt, ds(tg * 512, 512)],
                                start=(kt == 0),
                                stop=(kt == KT - 1),
                            )
                        if oft < 8:
                            nc.vector.tensor_scalar(
                                out=qT[:, oft, ds(tg * 512, 512)], in0=ps[:, :],
                                scalar1=bqt_sb[:, oft : oft + 1], scalar2=None,
                                op0=ALU.add,
                            )
                        else:
                            nc.vector.tensor_scalar(
                                out=kTt[:, oft - 8, ds(tg * 512, 512)], in0=ps[:, :],
                                scalar1=bkt_sb[:, oft - 8 : oft - 7], scalar2=None,
                                op0=ALU.add,
                            )

            with tc.tile_pool(name="wv", bufs=2) as wv_pool:
                for og in range(2):
                    wvt = wv_pool.tile([P, KT, 512], BF16, tag="wv", name="wvt")
                    for kt in range(KT):
                        nc.sync.dma_start(out=wvt[:, kt, :], in_=wv_d[kt, og])
                    for tt in range(NT):
                        ps = pt()
                        for kt in range(KT):
                            nc.tensor.matmul(
                                ps[:, :],
                                lhsT=h1T[:, kt, ts(tt, P)],
                                rhs=wvt[:, kt, :],
                                start=(kt == 0),
                                stop=(kt == KT - 1),
                            )
                        nc.vector.tensor_copy(
                            out=V1[:, tt, ds(og * 8, 8), 0:HD],
                            in_=ps[:, :].rearrange("p (h e) -> p h e", e=HD),
                        )

            ada_block(1, wada2_d)  # prefetch/compute overlapping QKV tail

            # ---------------- phase D: attention ----------------
            # Head pairs: even head on PE row-groups 0-1 (partitions 0:64),
            # odd head on row-groups 2-3 (partitions 64:128) -> interleaved
            # S^T matmuls run concurrently on the array.
            es_ctx = ExitStack()
            p_ctx = es_ctx.enter_context(
                tc.tile_pool(name="p_ctx", bufs=1, side="right")
            )
            ctxT = p_ctx.tile([P, KT, N], BF16, name="ctxT")

            with tc.tile_pool(name="etp", bufs=4) as et_pool, \
                 tc.tile_pool(name="ascr", bufs=2) as ascr:
                for hp in range(H // 2):
                    hf = hp
                    for qg in range(2):
                        ets = [
                            et_pool.tile([P, KT, 512], BF16, tag="et", name="et")
                            for _ in range(2)
                        ]
                        for kt in range(KT):
                            for sub in range(2):
                                m0 = 64 * sub
                                ps = pt()
                                nc.tensor.matmul(
                                    ps[:, :],
                                    lhsT=kTt[m0 : m0 + 64, hf, ts(kt, P)],
                                    rhs=qT[m0 : m0 + 64, hf, ds(qg * 512, 512)],
                                    start=True,
                                    stop=True,
                                )
                                nc.scalar.activation(
                                    out=ets[sub][:, kt, :], in_=ps[:, :],
                                    func=AF.Exp, scale=float(HD) ** -0.5,
                                )
                        for sub in range(2):
                            h = 2 * hp + sub
                            m0 = 64 * sub
                            et = ets[sub]
                            psc = pav()
                            for kt in range(KT):
                                nc.tensor.matmul(
                                    psc[0 : HD + 1, :],
                                    lhsT=V1[:, kt, h, :],
                                    rhs=et[:, kt, :],
                                    start=(kt == 0),
                                    stop=(kt == KT - 1),
                                )
                            rrow = ascr.tile([1, 512], FP32, tag="rrow", name="rrow")
                            nc.vector.reciprocal(
                                out=rrow[:, :], in_=psc[HD : HD + 1, :]
                            )
                            rbf = ascr.tile([1, 512], BF16, tag="rbf", name="rbf")
                            nc.vector.tensor_copy(out=rbf[:, :], in_=rrow[:, :])
                            psb = pt()
                            nc.tensor.matmul(
                                psb[0:HD, :],
                                lhsT=ones_bf[0:1, 0:HD],
                                rhs=rbf[0:1, :],
                                start=True,
                                stop=True,
                            )
                            rb = ascr.tile([HD, 512], FP32, tag="rb", name="rb")
                            nc.vector.tensor_copy(out=rb[:, :], in_=psb[0:HD, :])
                            ctmp = ascr.tile([HD, 512], FP32, tag="ctmp", name="ctmp")
                            nc.vector.tensor_tensor(
                                out=ctmp[:, :], in0=psc[0:HD, :], in1=rb[:, :],
                                op=ALU.mult,
                            )
                            if m0 == 0:
                                nc.vector.tensor_scalar(
                                    out=ctxT[0:HD, hf, ds(qg * 512, 512)],
                                    in0=ctmp[:, :],
                                    scalar1=bvt_sb[0:HD, hf : hf + 1], scalar2=None,
                                    op0=ALU.add,
                                )
                            else:
                                # DVE cannot shift partitions; stage at base 0
                                # then DMA-shift to partitions 64..127
                                cstg = ascr.tile(
                                    [HD, 512], BF16, tag="cstg", name="cstg"
                                )
                                nc.vector.tensor_scalar(
                                    out=cstg[:, :], in0=ctmp[:, :],
                                    scalar1=bvt_sb[m0 : m0 + HD, hf : hf + 1],
                                    scalar2=None, op0=ALU.add,
                                )
                                nc.sync.dma_start(
                                    out=ctxT[m0 : m0 + HD, hf, ds(qg * 512, 512)],
                                    in_=cstg[:, :],
                                )

            es_qkv.close()  # qT, kT, V1 dead
            es_h1.close()  # h1T dead

            # ---------------- phase E: proj + residual ----------------
            es_x1 = ExitStack()
            p_x1 = es_x1.enter_context(
                tc.tile_pool(name="p_x1", bufs=1, side="right")
            )
            x1_sb = p_x1.tile([P, NT, D], FP32, name="x1_sb")

            with tc.tile_pool(name="wp", bufs=2) as wp_pool, \
                 tc.tile_pool(name="escr", bufs=3) as escr, \
                 tc.tile_pool(name="sbE", bufs=1) as sbE:
                bpb = bcast_row(bproj_sb, "bpb", sbE)
                for og in range(2):
                    wpt = wp_pool.tile([P, KT, 512], BF16, tag="wp", name="wpt")
                    for kt in range(KT):
                        nc.sync.dma_start(out=wpt[:, kt, :], in_=wproj_d[kt, og])
                    for tt in range(NT):
                        ps = pt()
                        for kt in range(KT):
                            nc.tensor.matmul(
                                ps[:, :],
                                lhsT=ctxT[:, kt, ts(tt, P)],
                                rhs=wpt[:, kt, :],
                                start=(kt == 0),
                                stop=(kt == KT - 1),
                            )
                        t1 = escr.tile([P, 512], FP32, tag="t1", name="t1")
                        nc.vector.tensor_tensor(
                            out=t1[:, :], in0=ps[:, :], in1=bpb[:, ds(og * 512, 512)],
                            op=ALU.add,
                        )
                        nc.vector.tensor_tensor(
                            out=x1_sb[:, tt, ds(og * 512, 512)], in0=t1[:, :],
                            in1=x_sb[:, tt, ds(og * 512, 512)], op=ALU.add,
                        )

            es_x.close()  # x dead

            # ---------------- phase F: LN2 + transpose ----------------
            es_f = ExitStack()
            p_f = es_f.enter_context(tc.tile_pool(name="p_f", bufs=1, side="right"))
            fT = p_f.tile([P, 32, N], BF16, name="fT")
            es_h2 = ExitStack()
            p_h2 = es_h2.enter_context(
                tc.tile_pool(name="p_h2", bufs=1, side="right")
            )
            h2T = p_h2.tile([P, KT, N], BF16, name="h2T")
            with tc.tile_pool(name="sb2", bufs=1) as sb2, \
                 tc.tile_pool(name="lnscr2", bufs=2) as scr2:
                s2b = bcast_row(ss_bf[1][0], "s2b", sb2)
                sh2b = bcast_row(ss_bf[1][1], "sh2b", sb2)
                ln_transpose(x1_sb, s2b, sh2b, h2T, scr2)

            # ---------------- phase G: fc1 + gelu ----------------

            with tc.tile_pool(name="w1", bufs=3) as w1_pool, \
                 tc.tile_pool(name="gscr", bufs=2) as gscr:
                for oft in range(32):
                    w1t = w1_pool.tile([P, KT, P], BF16, tag="w1", name="w1t")
                    for kt in range(KT):
                        nc.sync.dma_start(out=w1t[:, kt, :], in_=wfc1_d[kt, oft])
                    for tg in range(2):
                        ps = pt()
                        for kt in range(KT):
                            nc.tensor.matmul(
                                ps[:, :],
                                lhsT=w1t[:, kt, :],
                                rhs=h2T[:, kt, ds(tg * 512, 512)],
                                start=(kt == 0),
                                stop=(kt == KT - 1),
                            )
                        # u = psum + b ; f = (1+approx(u))*u
                        # (the 0.5 of exact gelu is folded into w_fc2)
                        u = gscr.tile([P, 512], FP32, tag="u", name="u")
                        nc.vector.tensor_scalar(
                            out=u[:, :], in0=ps[:, :],
                            scalar1=bfc1t_sb[:, oft : oft + 1], scalar2=None,
                            op0=ALU.add,
                        )
                        v = gscr.tile([P, 512], FP32, tag="v", name="v")
                        if GELU_MODE == "erf":
                            # v = erf(u / sqrt(2))
                            nc.scalar.activation(
                                out=v[:, :], in_=ps[:, :], func=AF.Erf,
                                scale=0.7071067811865476,
                                bias=bfc1ts_sb[:, oft : oft + 1],
                            )
                        else:
                            # v = tanh(sqrt(2/pi) * (u + 0.044715 u^3))
                            s = gscr.tile([P, 512], FP32, tag="s", name="s")
                            nc.scalar.activation(
                                out=s[:, :], in_=u[:, :], func=AF.Square
                            )
                            w_ = gscr.tile([P, 512], FP32, tag="w_", name="w_")
                            nc.vector.tensor_scalar(
                                out=w_[:, :], in0=s[:, :],
                                scalar1=0.044715 * 0.7978845608028654,
                                scalar2=0.7978845608028654,
                                op0=ALU.mult, op1=ALU.add,
                            )
                            z = gscr.tile([P, 512], FP32, tag="z", name="z")
                            nc.vector.tensor_tensor(
                                out=z[:, :], in0=w_[:, :], in1=u[:, :], op=ALU.mult
                            )
                            nc.scalar.activation(
                                out=v[:, :], in_=z[:, :], func=AF.Tanh
                            )
                        nc.vector.scalar_tensor_tensor(
                            out=fT[:, oft, ds(tg * 512, 512)], in0=v[:, :],
                            scalar=1.0, in1=u[:, :],
                            op0=ALU.add, op1=ALU.mult,
                        )

            es_h2.close()

            # ---------------- phase H: fc2 + residual ----------------
            with tc.tile_pool(name="w2", bufs=1) as w2_pool, \
                 tc.tile_pool(name="hscr", bufs=3) as hscr, \
                 tc.tile_pool(name="sbH", bufs=1) as sbH:
                b2b = bcast_row(bfc2_sb, "b2b", sbH)
                for og in range(2):
                    w2t = w2_pool.tile([P, 32, 512], BF16, tag="w2", name="w2t")
                    for kt in range(32):
                        nc.sync.dma_start(out=w2t[:, kt, :], in_=wfc2_d[kt, og])
                    for tt in range(NT):
                        ps = pt()
                        for kt in range(32):
                            nc.tensor.matmul(
                                ps[:, :],
                                lhsT=fT[:, kt, ts(tt, P)],
                                rhs=w2t[:, kt, :],
                                start=(kt == 0),
                                stop=(kt == 31),
                            )
                        t1 = hscr.tile([P, 512], FP32, tag="ht1", name="ht1")
                        nc.vector.tensor_tensor(
                            out=t1[:, :], in0=ps[:, :], in1=b2b[:, ds(og * 512, 512)],
                            op=ALU.add,
                        )
                        ot = hscr.tile([P, 512], FP32, tag="ot", name="ot")
                        nc.vector.tensor_tensor(
                            out=ot[:, :], in0=t1[:, :],
                            in1=x1_sb[:, tt, ds(og * 512, 512)], op=ALU.add,
                        )
                        nc.sync.dma_start(
                            out=out_d[ts(tt, P), ds(og * 512, 512)], in_=ot[:, :]
                        )

            es_f.close()
            es_x1.close()
            es_ctx.close()

    nc.compile()
    return nc, names


def _bf(a):
    return np.ascontiguousarray(np.asarray(a, dtype=np.float32)).astype(BF16_NP)


def _f32(a):
    return np.ascontiguousarray(np.asarray(a, dtype=np.float32))


def prep_shared(w):
    """Host-side weight retiling (shared across cores)."""
    wqkv = np.asarray(w["w_qkv"], np.float32)
    shared = {
        "wqk": _bf(wqkv[:, : 2 * D].reshape(KT, P, 16, P).transpose(0, 2, 1, 3)),
        "wv": _bf(wqkv[:, 2 * D :].reshape(KT, P, 2, 512).transpose(0, 2, 1, 3)),
        "wproj": _bf(
            np.asarray(w["w_proj"], np.float32)
            .reshape(KT, P, 2, 512).transpose(0, 2, 1, 3)
        ),
        "wada1": _bf(
            np.asarray(w["w_ada1"], np.float32)
            .reshape(KT, P, 4, 512).transpose(0, 2, 1, 3)
        ),
        "wada2": _bf(
            np.asarray(w["w_ada2"], np.float32)
            .reshape(KT, P, 4, 512).transpose(0, 2, 1, 3)
        ),
        "wfc1": _bf(
            np.asarray(w["w_fc1"], np.float32)
            .reshape(KT, P, 32, P).transpose(0, 2, 1, 3)
        ),
        "wfc2": _bf(
            (np.asarray(w["w_fc2"], np.float32) * 0.5)
            .reshape(32, P, 2, 512).transpose(0, 2, 1, 3)
        ),
        "bada1": _f32(w["b_ada1"]).reshape(1, 2 * D),
        "bada2": _f32(w["b_ada2"]).reshape(1, 2 * D),
        "bqt": _f32(np.asarray(w["b_qkv"], np.float32)[:D].reshape(KT, P).T),
        "bkt": _f32(np.asarray(w["b_qkv"], np.float32)[D : 2 * D].reshape(KT, P).T),
        "bvt": _f32(np.asarray(w["b_qkv"], np.float32)[2 * D :].reshape(KT, P).T),
        "bfc1t": _f32(np.asarray(w["b_fc1"], np.float32).reshape(32, P).T),
        "bprojbf": _bf(w["b_proj"]).reshape(1, D),
        "bfc2bf": _bf(w["b_fc2"]).reshape(1, D),
    }
    shared["bfc1ts"] = _f32(shared["bfc1t"] * 0.7071067811865476)
    return shared


def make_in_maps(inputs, names):
    x = np.asarray(inputs["x"], np.float32)
    cond = np.asarray(inputs["condition"], np.float32)
    shared = prep_shared(inputs)
    in_maps = []
    for b in range(B):
        m = {
            names["x"]: np.ascontiguousarray(x[b]),
            names["condt"]: _bf(cond[b].reshape(KT, P).T),
        }
        for k, v in shared.items():
            m[names[k]] = v
        in_maps.append(m)
    return in_maps


_CACHE = {}


def kernel(**inputs) -> np.ndarray:
    if "nc" not in _CACHE:
        _CACHE["nc"], _CACHE["names"] = build()
    nc, names = _CACHE["nc"], _CACHE["names"]
    from concourse.bass_utils import run_bass_kernel_spmd

    in_maps = make_in_maps(inputs, names)
    res = run_bass_kernel_spmd(nc, in_maps, core_ids=list(range(B)))
    out = np.stack([np.asarray(res.results[b][names["out"]]) for b in range(B)])
    return out.astype(np.float32)


if __name__ == "__main__":
    nc, names = build()
    print("built ok:", len(names), "tensors")


# revision 18
# speedup vs baseline: 1.1941x; 1.1941x over previous
"""DiT block kernel for Trainium2, SPMD data-parallel over batch across 8 NeuronCores.

Per-core computation (one batch element, N=1024 tokens, D=1024):
  adaLN1 -> qkv -> attention(16 heads, hd=64) -> proj + residual
  adaLN2 -> fc1 -> gelu(exact/erf) -> fc2 + residual

Layout strategy:
  - residual stream x kept token-major (tm) [tok_p, feat] fp32 in SBUF
  - LN stats via bn_stats along free dim; adaLN scale/shift broadcast across
    partitions via tiny PE ones-matmuls
  - normalized h transposed to feature-major (fm) bf16 via PE transpose
  - all big matmuls in bf16 (fp32 PSUM accumulation)
  - Q^T,K^T produced fm (weights stationary), V token-major (acts stationary)
    with a ones-column appended so the attention AV matmul also produces the
    softmax denominators (softmax computed without max subtraction - safe at
    these scales, exact same math as reference)
  - proj/fc2 act-stationary -> token-major outputs + fp32 residual adds
"""

import sys

if "/opt/trn_rl_repo" not in sys.path:
    sys.path.insert(0, "/opt/trn_rl_repo")

from contextlib import ExitStack

import ml_dtypes
import numpy as np

import concourse.bacc as bacc
import concourse.bass as bass
import concourse.mybir as mybir
import concourse.tile as tile
from concourse.bass import ds, ts
from concourse.masks import make_identity

FP32 = mybir.dt.float32
BF16 = mybir.dt.bfloat16
AF = mybir.ActivationFunctionType
ALU = mybir.AluOpType

B, N, D = 8, 1024, 1024
H, HD, DFF = 16, 64, 4096
P = 128
NT = N // P   # 8 token tiles
KT = D // P   # 8 feature k-tiles
EPS = 1e-6
# "erf": exact gelu via Erf activation (not implemented in CoreSim, HW ok)
# "tanh": tanh-approx gelu from Square+Tanh (CoreSim-compatible fallback)
GELU_MODE = "erf"

BF16_NP = ml_dtypes.bfloat16


def build():
    """Build the single-core program (same program on all 8 cores)."""
    nc = bacc.Bacc(None, target_bir_lowering=False, debug=False)
    names = {}

    with tile.TileContext(nc) as tc:
        with ExitStack() as root:
            dram = root.enter_context(tc.tile_pool(name="dram", bufs=1, space="DRAM"))

            def din(nm, shape, dt=BF16):
                t = dram.tile(shape, dt, kind="ExternalInput", name=nm)
                names[nm] = t.name
                return t

            x_d = din("x", [N, D], FP32)
            condt_d = din("condt", [P, KT])
            wqk_d = din("wqk", [KT, 16, P, P])
            wv_d = din("wv", [KT, 2, P, 512])
            wproj_d = din("wproj", [KT, 2, P, 512])
            wada1_d = din("wada1", [KT, 4, P, 512])
            wada2_d = din("wada2", [KT, 4, P, 512])
            wfc1_d = din("wfc1", [KT, 32, P, P])
            wfc2_d = din("wfc2", [32, 2, P, 512])
            bada1_d = din("bada1", [1, 2 * D], FP32)
            bada2_d = din("bada2", [1, 2 * D], FP32)
            bqt_d = din("bqt", [P, KT], FP32)
            bkt_d = din("bkt", [P, KT], FP32)
            bvt_d = din("bvt", [P, KT], FP32)
            bfc1t_d = din("bfc1t", [P, 32], FP32)
            bfc1ts_d = din("bfc1ts", [P, 32], FP32)
            bproj_d = din("bprojbf", [1, D])
            bfc2_d = din("bfc2bf", [1, D])
            out_d = dram.tile([N, D], FP32, kind="ExternalOutput", name="out")
            names["out"] = out_d.name

            # ---------------- constants / small inputs ----------------
            const = root.enter_context(tc.tile_pool(name="const", bufs=1))
            psum = root.enter_context(tc.tile_pool(name="psum", bufs=6, space="PSUM"))

            def pt(nm="ps"):
                return psum.tile([P, 512], FP32, tag="ps", name=nm, bufs=4)

            def pav(nm="pav"):
                return psum.tile([P, 512], FP32, tag="pav", name=nm, bufs=2)

            def pt_tr(nm="pstr"):
                return psum.tile([P, P], BF16, tag="pstr", name=nm, bufs=2)

            condt_sb = const.tile([P, KT], BF16, name="condt_sb")
            nc.sync.dma_start(out=condt_sb[:, :], in_=condt_d[:, :])
            bqt_sb = const.tile([P, KT], FP32, name="bqt_sb")
            nc.sync.dma_start(out=bqt_sb[:, :], in_=bqt_d[:, :])
            bkt_sb = const.tile([P, KT], FP32, name="bkt_sb")
            nc.sync.dma_start(out=bkt_sb[:, :], in_=bkt_d[:, :])
            bvt_sb = const.tile([P, KT], FP32, name="bvt_sb")
            nc.sync.dma_start(out=bvt_sb[:, :], in_=bvt_d[:, :])
            bfc1t_sb = const.tile([P, 32], FP32, name="bfc1t_sb")
            nc.sync.dma_start(out=bfc1t_sb[:, :], in_=bfc1t_d[:, :])
            bfc1ts_sb = const.tile([P, 32], FP32, name="bfc1ts_sb")
            nc.sync.dma_start(out=bfc1ts_sb[:, :], in_=bfc1ts_d[:, :])
            bproj_sb = const.tile([1, D], BF16, name="bproj_sb")
            nc.sync.dma_start(out=bproj_sb[:, :], in_=bproj_d[:, :])
            bfc2_sb = const.tile([1, D], BF16, name="bfc2_sb")
            nc.sync.dma_start(out=bfc2_sb[:, :], in_=bfc2_d[:, :])
            bada_sb = []
            for ai, bd in enumerate((bada1_d, bada2_d)):
                t = const.tile([1, 2 * D], FP32, name=f"bada{ai}_sb")
                nc.sync.dma_start(out=t[:, :], in_=bd[:, :])
                bada_sb.append(t)

            ones_bf = const.tile([1, P], BF16, name="ones_bf")
            nc.vector.memset(ones_bf[:, :], 1.0)
            ident_bf = const.tile([P, P], BF16, name="ident_bf")
            make_identity(nc, ident_bf[:, :])
            zero_col = const.tile([P, 1], FP32, name="zero_col")
            nc.vector.memset(zero_col[:, :], 0.0)
            nc.const_aps.aps[(FP32, 0.0)] = zero_col[:, :]
            eps_col = const.tile([P, 1], FP32, name="eps_col")
            nc.vector.memset(eps_col[:, :], EPS)

            # ---------------- adaLN scale/shift rows ----------------
            ss_bf = []  # per ada: ((1+scale) bf16 row, shift bf16 row)

            def ada_block(ai, wada_d):
                with tc.tile_pool(name=f"wada{ai}", bufs=3) as wada_pool:
                    ss = const.tile([1, 2 * D], FP32, name=f"ss{ai}")
                    for og in range(4):
                        ps = pt()
                        for kt in range(KT):
                            wt = wada_pool.tile(
                                [P, 512], BF16, tag="wada", name="wadat"
                            )
                            nc.sync.dma_start(out=wt[:, :], in_=wada_d[kt, og])
                            nc.tensor.matmul(
                                ps[0:1, :],
                                lhsT=condt_sb[:, kt : kt + 1],
                                rhs=wt[:, :],
                                start=(kt == 0),
                                stop=(kt == KT - 1),
                            )
                        nc.vector.tensor_tensor(
                            out=ss[:, ds(og * 512, 512)],
                            in0=ps[0:1, :],
                            in1=bada_sb[ai][:, ds(og * 512, 512)],
                            op=ALU.add,
                        )
                    s_bf = const.tile([1, D], BF16, name=f"sbf{ai}")
                    nc.vector.tensor_scalar(
                        out=s_bf[:, :], in0=ss[:, 0:D], scalar1=1.0, scalar2=None,
                        op0=ALU.add,
                    )
                    sh_bf = const.tile([1, D], BF16, name=f"shbf{ai}")
                    nc.vector.tensor_copy(out=sh_bf[:, :], in_=ss[:, D : 2 * D])
                    ss_bf.append((s_bf, sh_bf))

            ada_block(0, wada1_d)

            def bcast_row(row_bf, nm, pool):
                """[1, D] bf16 row -> [128, D] fp32 (PE ones-matmul broadcast)."""
                outt = pool.tile([P, D], FP32, name=nm)
                for og in range(2):
                    ps = pt()
                    nc.tensor.matmul(
                        ps[:, :],
                        lhsT=ones_bf[0:1, :],
                        rhs=row_bf[0:1, ds(og * 512, 512)],
                        start=True,
                        stop=True,
                    )
                    nc.vector.tensor_copy(out=outt[:, ds(og * 512, 512)], in_=ps[:, :])
                return outt

            def ln_transpose(x_t, s_b, sh_b, hT, scr):
                """LayerNorm(x_t)*(s_b) + sh_b, then transpose to fm bf16 hT."""
                for tt in range(NT):
                    xt = x_t[:, tt, :]
                    st = scr.tile([P, 2, 6], FP32, tag="bnst", name="bnst")
                    xr = xt.rearrange("p (s f) -> p s f", f=512)
                    for sg in range(2):
                        nc.vector.bn_stats(out=st[:, sg, :], in_=xr[:, sg, :])
                    mv = scr.tile([P, 2], FP32, tag="bnmv", name="bnmv")
                    nc.vector.bn_aggr(out=mv[:, :], in_=st[:, :, :])
                    nc.scalar.activation(
                        out=mv[:, 1:2], in_=mv[:, 1:2], func=AF.Sqrt,
                        bias=eps_col[:, 0:1],
                    )
                    nc.vector.reciprocal(out=mv[:, 1:2], in_=mv[:, 1:2])
                    xn = scr.tile([P, D], FP32, tag="xn", name="xn")
                    nc.vector.tensor_scalar(
                        out=xn[:, :], in0=xt, scalar1=mv[:, 0:1], scalar2=mv[:, 1:2],
                        op0=ALU.subtract, op1=ALU.mult,
                    )
                    t2 = scr.tile([P, D], FP32, tag="t2", name="t2")
                    nc.vector.tensor_tensor(
                        out=t2[:, :], in0=xn[:, :], in1=s_b[:, :], op=ALU.mult
                    )
                    hbf = scr.tile([P, D], BF16, tag="hbf", name="hbf")
                    nc.vector.tensor_tensor(
                        out=hbf[:, :], in0=t2[:, :], in1=sh_b[:, :], op=ALU.add
                    )
                    for ft in range(KT):
                        ps = pt_tr()
                        nc.tensor.transpose(
                            ps[:, :], hbf[:, ts(ft, P)], ident_bf[:, :]
                        )
                        nc.vector.tensor_copy(
                            out=hT[:, ft, ts(tt, P)], in_=ps[:, :]
                        )

            # ---------------- load x ----------------
            es_x = ExitStack()
            p_x = es_x.enter_context(tc.tile_pool(name="p_x", bufs=1))
            x_sb = p_x.tile([P, NT, D], FP32, name="x_sb")
            for tt in range(NT):
                nc.sync.dma_start(out=x_sb[:, tt, :], in_=x_d[ts(tt, P), :])

            # ---------------- phase B: LN1 + transpose ----------------
            es_h1 = ExitStack()
            p_h1 = es_h1.enter_context(tc.tile_pool(name="p_h1", bufs=1))
            h1T = p_h1.tile([P, KT, N], BF16, name="h1T")
            with tc.tile_pool(name="lnscr1", bufs=2) as scr1, \
                 tc.tile_pool(name="sb1", bufs=1) as sb1:
                s1b = bcast_row(ss_bf[0][0], "s1b", sb1)
                sh1b = bcast_row(ss_bf[0][1], "sh1b", sb1)
                ln_transpose(x_sb, s1b, sh1b, h1T, scr1)

            # ---------------- phase C: QKV ----------------
            es_qkv = ExitStack()
            p_qkv = es_qkv.enter_context(tc.tile_pool(name="p_qkv", bufs=1))
            qT = p_qkv.tile([P, KT, N], BF16, name="qT")
            kTt = p_qkv.tile([P, KT, N], BF16, name="kTt")
            V1 = p_qkv.tile([P, NT, H, HD + 1], BF16, name="V1")
            nc.vector.memset(V1[:, :, :, HD : HD + 1], 1.0)

            with tc.tile_pool(name="wqk", bufs=3) as wqk_pool:
                for oft in range(16):
                    wt = wqk_pool.tile([P, KT, P], BF16, tag="wqk", name="wqkt")
                    for kt in range(KT):
                        nc.sync.dma_start(out=wt[:, kt, :], in_=wqk_d[kt, oft])
                    for tg in range(2):
                        ps = pt()
                        for kt in range(KT):
                            nc.tensor.matmul(
                                ps[:, :],
                                lhsT=wt[:, kt, :],
                                rhs=h1T[:, kt, ds(tg * 512, 512)],
                                start=(kt == 0),
                                stop=(kt == KT - 1),
                            )
                        if oft < 8:
                            nc.vector.tensor_scalar(
                                out=qT[:, oft, ds(tg * 512, 512)], in0=ps[:, :],
                                scalar1=bqt_sb[:, oft : oft + 1], scalar2=None,
                                op0=ALU.add,
                            )
                        else:
                            nc.vector.tensor_scalar(
                                out=kTt[:, oft - 8, ds(tg * 512, 512)], in0=ps[:, :],
                                scalar1=bkt_sb[:, oft - 8 : oft - 7], scalar2=None,
                                op0=ALU.add,
                            )

            with tc.tile_pool(name="wv", bufs=2) as wv_pool:
                for og in range(2):
                    wvt = wv_pool.tile([P, KT, 512], BF16, tag="wv", name="wvt")
                    for kt in range(KT):
                        nc.sync.dma_start(out=wvt[:, kt, :], in_=wv_d[kt, og])
                    for tt in range(NT):
                        ps = pt()
                        for kt in range(KT):
                            nc.tensor.matmul(
                                ps[:, :],
                                lhsT=h1T[:, kt, ts(tt, P)],
                                rhs=wvt[:, kt, :],
                                start=(kt == 0),
                                stop=(kt == KT - 1),
                            )
                        nc.vector.tensor_copy(
                            out=V1[:, tt, ds(og * 8, 8), 0:HD],
                            in_=ps[:, :].rearrange("p (h e) -> p h e", e=HD),
                        )

            ada_block(1, wada2_d)  # prefetch/compute overlapping QKV tail

            # ---------------- phase D: attention ----------------
            # Head pairs: even head on PE row-groups 0-1 (partitions 0:64),
            # odd head on row-groups 2-3 (partitions 64:128) -> interleaved
            # S^T matmuls run concurrently on the array.
            es_ctx = ExitStack()
            p_ctx = es_ctx.enter_context(
                tc.tile_pool(name="p_ctx", bufs=1, side="right")
            )
            ctxT = p_ctx.tile([P, KT, N], BF16, name="ctxT")

            units = [(h, qg) for h in range(H) for qg in range(2)]
            DEPTH = 2  # units of S/exp emitted ahead of each AV

            with tc.tile_pool(name="etp", bufs=4) as et_pool, \
                 tc.tile_pool(name="ascr", bufs=2) as ascr:

                def emit_S(h, qg, et):
                    m0 = 64 * (h % 2)
                    hf = h // 2
                    for kt in range(KT):
                        ps = pt()
                        nc.tensor.matmul(
                            ps[:, :],
                            lhsT=kTt[m0 : m0 + 64, hf, ts(kt, P)],
                            rhs=qT[m0 : m0 + 64, hf, ds(qg * 512, 512)],
                            start=True,
                            stop=True,
                        )
                        nc.scalar.activation(
                            out=et[:, kt, :], in_=ps[:, :], func=AF.Exp,
                            scale=float(HD) ** -0.5,
                        )

                def emit_AV(h, qg, et):
                    m0 = 64 * (h % 2)
                    hf = h // 2
                    psc = pav()
                    for kt in range(KT):
                        nc.tensor.matmul(
                            psc[0 : HD + 1, :],
                            lhsT=V1[:, kt, h, :],
                            rhs=et[:, kt, :],
                            start=(kt == 0),
                            stop=(kt == KT - 1),
                        )
                    rrow = ascr.tile([1, 512], FP32, tag="rrow", name="rrow")
                    nc.vector.reciprocal(out=rrow[:, :], in_=psc[HD : HD + 1, :])
                    rbf = ascr.tile([1, 512], BF16, tag="rbf", name="rbf")
                    nc.vector.tensor_copy(out=rbf[:, :], in_=rrow[:, :])
                    psb = pt()
                    nc.tensor.matmul(
                        psb[0:HD, :],
                        lhsT=ones_bf[0:1, 0:HD],
                        rhs=rbf[0:1, :],
                        start=True,
                        stop=True,
                    )
                    rb = ascr.tile([HD, 512], FP32, tag="rb", name="rb")
                    nc.vector.tensor_copy(out=rb[:, :], in_=psb[0:HD, :])
                    ctmp = ascr.tile([HD, 512], FP32, tag="ctmp", name="ctmp")
                    nc.vector.tensor_tensor(
                        out=ctmp[:, :], in0=psc[0:HD, :], in1=rb[:, :], op=ALU.mult
                    )
                    if m0 == 0:
                        nc.vector.tensor_scalar(
                            out=ctxT[0:HD, hf, ds(qg * 512, 512)], in0=ctmp[:, :],
                            scalar1=bvt_sb[0:HD, hf : hf + 1], scalar2=None,
                            op0=ALU.add,
                        )
                    else:
                        # DVE cannot shift partitions; stage at base 0 then
                        # DMA-shift to partitions 64..127
                        cstg = ascr.tile([HD, 512], BF16, tag="cstg", name="cstg")
                        nc.vector.tensor_scalar(
                            out=cstg[:, :], in0=ctmp[:, :],
                            scalar1=bvt_sb[m0 : m0 + HD, hf : hf + 1],
                            scalar2=None, op0=ALU.add,
                        )
                        nc.sync.dma_start(
                            out=ctxT[m0 : m0 + HD, hf, ds(qg * 512, 512)],
                            in_=cstg[:, :],
                        )

                ets = {}
                for i in range(len(units) + DEPTH):
                    if i < len(units):
                        ets[i] = et_pool.tile(
                            [P, KT, 512], BF16, tag="et", name="et"
                        )
                        emit_S(*units[i], ets[i])
                    j = i - DEPTH
                    if j >= 0:
                        emit_AV(*units[j], ets.pop(j))

            es_qkv.close()  # qT, kT, V1 dead
            es_h1.close()  # h1T dead

            # ---------------- phase E: proj + residual ----------------
            es_x1 = ExitStack()
            p_x1 = es_x1.enter_context(
                tc.tile_pool(name="p_x1", bufs=1, side="right")
            )
            x1_sb = p_x1.tile([P, NT, D], FP32, name="x1_sb")

            with tc.tile_pool(name="wp", bufs=2) as wp_pool, \
                 tc.tile_pool(name="escr", bufs=3) as escr, \
                 tc.tile_pool(name="sbE", bufs=1) as sbE:
                bpb = bcast_row(bproj_sb, "bpb", sbE)
                for og in range(2):
                    wpt = wp_pool.tile([P, KT, 512], BF16, tag="wp", name="wpt")
                    for kt in range(KT):
                        nc.sync.dma_start(out=wpt[:, kt, :], in_=wproj_d[kt, og])
                    for tt in range(NT):
                        ps = pt()
                        for kt in range(KT):
                            nc.tensor.matmul(
                                ps[:, :],
                                lhsT=ctxT[:, kt, ts(tt, P)],
                                rhs=wpt[:, kt, :],
                                start=(kt == 0),
                                stop=(kt == KT - 1),
                            )
                        t1 = escr.tile([P, 512], FP32, tag="t1", name="t1")
                        nc.vector.tensor_tensor(
                            out=t1[:, :], in0=ps[:, :], in1=bpb[:, ds(og * 512, 512)],
                            op=ALU.add,
                        )
                        nc.vector.tensor_tensor(
                            out=x1_sb[:, tt, ds(og * 512, 512)], in0=t1[:, :],
                            in1=x_sb[:, tt, ds(og * 512, 512)], op=ALU.add,
                        )

            es_x.close()  # x dead

            # ---------------- phase F: LN2 + transpose ----------------
            es_f = ExitStack()
            p_f = es_f.enter_context(tc.tile_pool(name="p_f", bufs=1, side="right"))
            fT = p_f.tile([P, 32, N], BF16, name="fT")
            es_h2 = ExitStack()
            p_h2 = es_h2.enter_context(
                tc.tile_pool(name="p_h2", bufs=1, side="right")
            )
            h2T = p_h2.tile([P, KT, N], BF16, name="h2T")
            with tc.tile_pool(name="sb2", bufs=1) as sb2, \
                 tc.tile_pool(name="lnscr2", bufs=2) as scr2:
                s2b = bcast_row(ss_bf[1][0], "s2b", sb2)
                sh2b = bcast_row(ss_bf[1][1], "sh2b", sb2)
                ln_transpose(x1_sb, s2b, sh2b, h2T, scr2)

            # ---------------- phase G: fc1 + gelu ----------------

            with tc.tile_pool(name="w1", bufs=3) as w1_pool, \
                 tc.tile_pool(name="gscr", bufs=2) as gscr:
                for oft in range(32):
                    w1t = w1_pool.tile([P, KT, P], BF16, tag="w1", name="w1t")
                    for kt in range(KT):
                        nc.sync.dma_start(out=w1t[:, kt, :], in_=wfc1_d[kt, oft])
                    for tg in range(2):
                        ps = pt()
                        for kt in range(KT):
                            nc.tensor.matmul(
                                ps[:, :],
                                lhsT=w1t[:, kt, :],
                                rhs=h2T[:, kt, ds(tg * 512, 512)],
                                start=(kt == 0),
                                stop=(kt == KT - 1),
                            )
                        # u = psum + b ; f = (1+approx(u))*u
                        # (the 0.5 of exact gelu is folded into w_fc2)
                        u = gscr.tile([P, 512], FP32, tag="u", name="u")
                        nc.vector.tensor_scalar(
                            out=u[:, :], in0=ps[:, :],
                            scalar1=bfc1t_sb[:, oft : oft + 1], scalar2=None,
                            op0=ALU.add,
                        )
                        v = gscr.tile([P, 512], FP32, tag="v", name="v")
                        if GELU_MODE == "erf":
                            # v = erf(u / sqrt(2))
                            nc.scalar.activation(
                                out=v[:, :], in_=ps[:, :], func=AF.Erf,
                                scale=0.7071067811865476,
                                bias=bfc1ts_sb[:, oft : oft + 1],
                            )
                        else:
                            # v = tanh(sqrt(2/pi) * (u + 0.044715 u^3))
                            s = gscr.tile([P, 512], FP32, tag="s", name="s")
                            nc.scalar.activation(
                                out=s[:, :], in_=u[:, :], func=AF.Square
                            )
                            w_ = gscr.tile([P, 512], FP32, tag="w_", name="w_")
                            nc.vector.tensor_scalar(
                                out=w_[:, :], in0=s[:, :],
                                scalar1=0.044715 * 0.7978845608028654,
                                scalar2=0.7978845608028654,
                                op0=ALU.mult, op1=ALU.add,
                            )
                            z = gscr.tile([P, 512], FP32, tag="z", name="z")
                            nc.vector.tensor_tensor(
                                out=z[:, :], in0=w_[:, :], in1=u[:, :], op=ALU.mult
                            )
                            nc.scalar.activation(
                                out=v[:, :], in_=z[:, :], func=AF.Tanh
                            )
                        nc.vector.scalar_tensor_tensor(
                            out=fT[:, oft, ds(tg * 512, 512)], in0=v[:, :],
                            scalar=1.0, in1=u[:, :],
                            op0=ALU.add, op1=ALU.mult,
                        )

            es_h2.close()

            # ---------------- phase H: fc2 + residual ----------------
            with tc.tile_pool(name="w2", bufs=1) as w2_pool, \
                 tc.tile_pool(name="hscr", bufs=3) as hscr, \
                 tc.tile_pool(name="sbH", bufs=1) as sbH:
                b2b = bcast_row(bfc2_sb, "b2b", sbH)
                for og in range(2):
                    w2t = w2_pool.tile([P, 32, 512], BF16, tag="w2", name="w2t")
                    for kt in range(32):
                        nc.sync.dma_start(out=w2t[:, kt, :], in_=wfc2_d[kt, og])
                    for tt in range(NT):
                        ps = pt()
                        for kt in range(32):
                            nc.tensor.matmul(
                                ps[:, :],
                                lhsT=fT[:, kt, ts(tt, P)],
                                rhs=w2t[:, kt, :],
                                start=(kt == 0),
                                stop=(kt == 31),
                            )
                        t1 = hscr.tile([P, 512], FP32, tag="ht1", name="ht1")
                        nc.vector.tensor_tensor(
                            out=t1[:, :], in0=ps[:, :], in1=b2b[:, ds(og * 512, 512)],
                            op=ALU.add,
                        )
                        ot = hscr.tile([P, 512], FP32, tag="ot", name="ot")
                        nc.vector.tensor_tensor(
                            out=ot[:, :], in0=t1[:, :],
                            in1=x1_sb[:, tt, ds(og * 512, 512)], op=ALU.add,
                        )
                        nc.sync.dma_start(
                            out=out_d[ts(tt, P), ds(og * 512, 512)], in_=ot[:, :]
                        )

            es_f.close()
            es_x1.close()
            es_ctx.close()

    nc.compile()
    return nc, names


def _bf(a):
    return np.ascontiguousarray(np.asarray(a, dtype=np.float32)).astype(BF16_NP)


def _f32(a):
    return np.ascontiguousarray(np.asarray(a, dtype=np.float32))


def prep_shared(w):
    """Host-side weight retiling (shared across cores)."""
    wqkv = np.asarray(w["w_qkv"], np.float32)
    shared = {
        "wqk": _bf(wqkv[:, : 2 * D].reshape(KT, P, 16, P).transpose(0, 2, 1, 3)),
        "wv": _bf(wqkv[:, 2 * D :].reshape(KT, P, 2, 512).transpose(0, 2, 1, 3)),
        "wproj": _bf(
            np.asarray(w["w_proj"], np.float32)
            .reshape(KT, P, 2, 512).transpose(0, 2, 1, 3)
        ),
        "wada1": _bf(
            np.asarray(w["w_ada1"], np.float32)
            .reshape(KT, P, 4, 512).transpose(0, 2, 1, 3)
        ),
        "wada2": _bf(
            np.asarray(w["w_ada2"], np.float32)
            .reshape(KT, P, 4, 512).transpose(0, 2, 1, 3)
        ),
        "wfc1": _bf(
            np.asarray(w["w_fc1"], np.float32)
            .reshape(KT, P, 32, P).transpose(0, 2, 1, 3)
        ),
        "wfc2": _bf(
            (np.asarray(w["w_fc2"], np.float32) * 0.5)
            .reshape(32, P, 2, 512).transpose(0, 2, 1, 3)
        ),
        "bada1": _f32(w["b_ada1"]).reshape(1, 2 * D),
        "bada2": _f32(w["b_ada2"]).reshape(1, 2 * D),
        "bqt": _f32(np.asarray(w["b_qkv"], np.float32)[:D].reshape(KT, P).T),
        "bkt": _f32(np.asarray(w["b_qkv"], np.float32)[D : 2 * D].reshape(KT, P).T),
        "bvt": _f32(np.asarray(w["b_qkv"], np.float32)[2 * D :].reshape(KT, P).T),
        "bfc1t": _f32(np.asarray(w["b_fc1"], np.float32).reshape(32, P).T),
        "bprojbf": _bf(w["b_proj"]).reshape(1, D),
        "bfc2bf": _bf(w["b_fc2"]).reshape(1, D),
    }
    shared["bfc1ts"] = _f32(shared["bfc1t"] * 0.7071067811865476)
    return shared


def make_in_maps(inputs, names):
    x = np.asarray(inputs["x"], np.float32)
    cond = np.asarray(inputs["condition"], np.float32)
    shared = prep_shared(inputs)
    in_maps = []
    for b in range(B):
        m = {
            names["x"]: np.ascontiguousarray(x[b]),
            names["condt"]: _bf(cond[b].reshape(KT, P).T),
        }
        for k, v in shared.items():
            m[names[k]] = v
        in_maps.append(m)
    return in_maps


_CACHE = {}


def kernel(**inputs) -> np.ndarray:
    if "nc" not in _CACHE:
        _CACHE["nc"], _CACHE["names"] = build()
    nc, names = _CACHE["nc"], _CACHE["names"]
    from concourse.bass_utils import run_bass_kernel_spmd

    in_maps = make_in_maps(inputs, names)
    res = run_bass_kernel_spmd(nc, in_maps, core_ids=list(range(B)))
    out = np.stack([np.asarray(res.results[b][names["out"]]) for b in range(B)])
    return out.astype(np.float32)


if __name__ == "__main__":
    nc, names = build()
    print("built ok:", len(names), "tensors")
